# revision 1
# baseline (speedup 1.0000x reference)
"""Trainium2 Bass kernel for a fused autoregressive tanh-RNN decoder.

Model (per step t):
    h = tanh(x @ W_ih.T + b_ih + h @ W_hh.T + b_hh)   # h: [B,H], x: [B,1]
    y = h @ W_out.T + b_out                           # [B,1]
    x = tf[t] ? targets[t] : y
with T=256 steps, B=512, H=2048.

Sharding: data-parallel over batch — 64 rows per core on 8 cores; weights
replicated. The scan carry stays core-local so there is no per-step
communication.

Per-core kernel structure (fp16 matmul operands, fp32 PSUM accumulate):
  * The hidden state is kept TRANSPOSED (h^T, [H on partitions x 64 batch])
    and used as the matmul *stationary* operand; the big W_hh streams through
    the PE as the moving operand. This keeps LDWEIGHTS traffic tiny (64-col
    loads) and runs the PE at its streaming floor.
  * The 128x128 array is column-split into two halves (tile_position col 0 /
    col 64) producing the two H/2 output halves of each chunk.
  * x @ W_ih.T + (b_ih+b_hh) is folded into the same PSUM accumulation as an
    extra 2-row stationary pass ([x^T; ones] against [W_ih^T; bias]).
  * tanh on ScalarE (PSUM -> SBUF fp16); y = h.W_out via a fused DVE
    multiply+free-dim-reduce (scalar_tensor_tensor accum), a tiny PE
    transpose of the 128 partials, and a half-fold; teacher-force select via
    copy_predicated in row layout; the next step's stationary h^T is rebuilt
    with 16 PE transposes.
"""

import numpy as np

T, B, H = 256, 512, 2048
NCORES = 8
BC = B // NCORES          # 64 batch rows per core
JT = H // 128             # 16 contraction (K) tiles
HH = H // 2               # 1024, per-partition-half output columns

_CACHE = {}


TRANSPOSE_VIA_DMA = False
EXPLICIT_LDW = False

ORDER_4SHARE = False
SPLIT_BANKS = False


def _build_program(n_steps, repeat=1):
    import concourse.bass as bass
    import concourse.tile as tile
    from concourse import bacc, mybir

    fp16 = mybir.dt.float16
    fp32 = mybir.dt.float32
    u8 = mybir.dt.uint8
    Tanh = mybir.ActivationFunctionType.Tanh
    mult = mybir.AluOpType.mult
    add = mybir.AluOpType.add

    nc = bacc.Bacc("TRN2", target_bir_lowering=False, debug=False,
                   num_devices=NCORES)

    d_W = nc.dram_tensor("w_rhs", [128, JT * H], fp16, kind="ExternalInput")
    d_Wx = nc.dram_tensor("w_x", [2, H], fp16, kind="ExternalInput")
    d_Wout = nc.dram_tensor("w_out_rep", [128, HH], fp16, kind="ExternalInput")
    d_h0 = nc.dram_tensor("h0t", [128, JT * BC], fp16, kind="ExternalInput")
    d_x0 = nc.dram_tensor("x0t", [2, BC], fp16, kind="ExternalInput")
    d_tgt = nc.dram_tensor("tgt16", [1, T * BC], fp16, kind="ExternalInput")
    d_tf = nc.dram_tensor("tfmask", [1, T * BC], u8, kind="ExternalInput")
    d_bout = nc.dram_tensor("bout_s", [1, 1], fp32, kind="ExternalInput")
    d_idf = nc.dram_tensor("identf", [128, 128], fp32, kind="ExternalInput")
    d_id = nc.dram_tensor("ident", [128, BC], fp16, kind="ExternalInput")
    d_y = nc.dram_tensor("y_out", [1, T * BC], fp32, kind="ExternalOutput")

    with tile.TileContext(nc) as tc:
        with (
            tc.tile_pool(name="const", bufs=1) as constp,
            tc.tile_pool(name="stat", bufs=2) as statp,
            tc.tile_pool(name="hbuf", bufs=2) as hbufp,
            tc.tile_pool(name="scr", bufs=2) as scrp,
            tc.tile_pool(name="small", bufs=3) as smallp,
            tc.tile_pool(name="psmain", bufs=1 if SPLIT_BANKS else 2, space="PSUM") as psmainp,
            tc.tile_pool(name="pstr", bufs=2, space="PSUM") as pstrp,
            tc.tile_pool(name="psx", bufs=2, space="PSUM") as psxp,
            tc.tile_pool(name="psy", bufs=2, space="PSUM") as psyp,
        ):
            # --- persistent SBUF residents -------------------------------
            sb_W = constp.tile([128, JT * H], fp16)
            nc.sync.dma_start(sb_W[:], d_W.ap())
            sb_Wx = constp.tile([2, H], fp16)
            nc.sync.dma_start(sb_Wx[:], d_Wx.ap())
            sb_Wout = constp.tile([128, HH], fp16)
            nc.sync.dma_start(sb_Wout[:], d_Wout.ap())
            sb_tgt = constp.tile([1, T * BC], fp16)
            nc.sync.dma_start(sb_tgt[:], d_tgt.ap())
            sb_tf = constp.tile([1, T * BC], u8)
            nc.sync.dma_start(sb_tf[:], d_tf.ap())
            sb_bout = constp.tile([1, 1], fp32)
            nc.sync.dma_start(sb_bout[:], d_bout.ap())
            sb_idf = constp.tile([128, 128], fp32)
            nc.sync.dma_start(sb_idf[:], d_idf.ap())
            sb_id = constp.tile([128, BC], fp16)
            nc.sync.dma_start(sb_id[:], d_id.ap())
            sb_y = constp.tile([1, T * BC], fp32)
            nc.vector.memset(sb_y[:], 0.0)
            # [x^T; ones] stationary rows; row 0 is rewritten each step.
            sb_xstat = constp.tile([2, BC], fp16)
            nc.sync.dma_start(sb_xstat[:], d_x0.ap())

            stat = statp.tile([128, JT * BC], fp16)
            nc.sync.dma_start(stat[:], d_h0.ap())

            # PSUM slot for (half, cp): with SPLIT_BANKS each of the four
            # accumulation regions sits in its own PSUM bank.
            def ps_slice(ps, half, cp):
                if SPLIT_BANKS:
                    bank = 2 * cp + half
                    return ps[64 * half:64 * half + 64,
                              bank * 512:(bank + 1) * 512]
                return ps[64 * half:64 * half + 64, cp * 512:(cp + 1) * 512]

            for rep in range(repeat):
              for t in range(n_steps):
                ps = psmainp.tile([128, (4 if SPLIT_BANKS else 2) * 512], fp32)
                sb_h = hbufp.tile([128, HH], fp16)

                def main_mm(half, cp, j):
                    nc.tensor.matmul(
                        ps_slice(ps, half, cp),
                        stat[:, j * BC:(j + 1) * BC],
                        sb_W[:, j * H + half * HH + cp * 512:
                             j * H + half * HH + (cp + 1) * 512],
                        start=(j == 0), stop=False,
                        skip_group_check=True,
                    )

                def pass17(half, cp):
                    nc.tensor.matmul(
                        ps_slice(ps, half, cp),
                        sb_xstat[:],
                        sb_Wx[:, half * HH + cp * 512:
                              half * HH + (cp + 1) * 512],
                        start=False, stop=True,
                        skip_group_check=True,
                    )

                def tanh_cp(cp):
                    if SPLIT_BANKS:
                        for half in (0, 1):
                            nc.scalar.activation(
                                sb_h[64 * half:64 * half + 64,
                                     cp * 512:(cp + 1) * 512],
                                ps_slice(ps, half, cp),
                                Tanh,
                            )
                    else:
                        nc.scalar.activation(
                            sb_h[:, cp * 512:(cp + 1) * 512],
                            ps[:, cp * 512:(cp + 1) * 512],
                            Tanh,
                        )

                if ORDER_4SHARE:
                    # per stationary: 4 MMs differing in col-group and bank
                    for j in range(JT):
                        if EXPLICIT_LDW:
                            for half in (0, 1):
                                nc.tensor.ldweights(
                                    stat[:, j * BC:(j + 1) * BC],
                                    tile_position=(0, 64 * half),
                                )
                        for half, cp in ((0, 0), (1, 1), (0, 1), (1, 0)):
                            main_mm(half, cp, j)
                    for cp in (0, 1):
                        for half in (0, 1):
                            pass17(half, cp)
                        tanh_cp(cp)
                else:
                    for cp in (0, 1):
                        for j in range(JT):
                            if EXPLICIT_LDW:
                                for half in (0, 1):
                                    nc.tensor.ldweights(
                                        stat[:, j * BC:(j + 1) * BC],
                                        tile_position=(0, 64 * half),
                                    )
                            for half in (0, 1):
                                main_mm(half, cp, j)
                        for half in (0, 1):
                            pass17(half, cp)
                        tanh_cp(cp)

                # rebuild the transposed stationary for the next step
                statn = statp.tile([128, JT * BC], fp16)
                if TRANSPOSE_VIA_DMA:
                    for j in (0, 1, 2, 3, 8, 9, 10, 11, 4, 5, 6, 7, 12, 13, 14, 15):
                        half, blk = divmod(j, 8)
                        nc.sync.dma_start(
                            statn[:, j * BC:(j + 1) * BC],
                            sb_h[64 * half:64 * half + 64,
                                 blk * 128:(blk + 1) * 128],
                            transpose=True,
                        )
                else:
                    for pair in (0, 1, 4, 5, 2, 3, 6, 7):
                        pt = pstrp.tile([128, 128], fp16)
                        for k2 in (0, 1):
                            j = 2 * pair + k2
                            half, blk = divmod(j, 8)
                            nc.tensor.transpose(
                                pt[:, 64 * k2:64 * k2 + 64],
                                sb_h[64 * half:64 * half + 64,
                                     blk * 128:(blk + 1) * 128],
                                sb_id[64 * half:64 * half + 64, 0:64],
                            )
                        nc.vector.tensor_copy(
                            statn[:, pair * 128:(pair + 1) * 128], pt[:])

                # y = h . W_out + b_out: free-dim reduce per partition, then
                # transpose the 128 partials to one row and fold the halves
                ypart = smallp.tile([128, 1], fp32, tag="ypart")
                scr = scrp.tile([128, HH], fp16)
                nc.vector.scalar_tensor_tensor(
                    out=scr[:], in0=sb_h[:], scalar=1.0, in1=sb_Wout[:],
                    op0=mult, op1=mult, accum_out=ypart[:],
                )
                psy = psyp.tile([1, 128], fp32, tag="ypart_t")
                nc.tensor.transpose(psy[:], ypart[:], sb_idf[:])
                yt = smallp.tile([1, 128], fp32, tag="yt")
                nc.vector.tensor_copy(yt[:], psy[:])
                nc.vector.scalar_tensor_tensor(
                    out=sb_y[:, t * BC:(t + 1) * BC],
                    in0=yt[0:1, 0:BC], scalar=sb_bout[:],
                    in1=yt[0:1, BC:128], op0=add, op1=add,
                )

                if t + 1 < n_steps or rep + 1 < repeat:
                    # x' = tf ? target : y  (row layout, no transpose needed)
                    x16 = smallp.tile([1, BC], fp16, tag="x16")
                    nc.vector.tensor_copy(x16[:], sb_y[:, t * BC:(t + 1) * BC])
                    nc.vector.copy_predicated(
                        x16[:], sb_tf[:, t * BC:(t + 1) * BC],
                        sb_tgt[:, t * BC:(t + 1) * BC])
                    nc.vector.tensor_copy(sb_xstat[0:1, :], x16[:])

                stat = statn

            nc.sync.dma_start(d_y.ap(), sb_y[:])

    nc.compile()
    return nc


def _prep_inputs(initial_input, hidden, targets, W_ih, b_ih, W_hh, b_hh,
                 W_out, b_out, tf_mask):
    f16 = np.float16
    # moving operand: W[d, j*H + i] = W_hh[i, 128j+d]
    w = np.ascontiguousarray(W_hh.T.astype(f16))              # [j, i]
    w = w.reshape(JT, 128, H).transpose(1, 0, 2).reshape(128, JT * H)
    wx = np.stack([W_ih[:, 0], (b_ih + b_hh)]).astype(f16)    # [2, H]
    wout = np.concatenate(
        [np.tile(W_out[0, :HH], (64, 1)), np.tile(W_out[0, HH:], (64, 1))],
        axis=0).astype(f16)                                   # [128, HH]
    identf = np.eye(128, dtype=np.float32)
    bout = np.full((1, 1), np.float32(b_out[0]), np.float32)
    tf_row = np.repeat(tf_mask.astype(np.uint8), BC)[None, :]  # [1, T*BC]

    ident = np.concatenate([np.eye(BC), np.eye(BC)], axis=0).astype(f16)
    shared = dict(w_rhs=np.ascontiguousarray(w), w_x=np.ascontiguousarray(wx),
                  w_out_rep=np.ascontiguousarray(wout), identf=identf,
                  ident=ident, bout_s=bout, tfmask=np.ascontiguousarray(tf_row))

    in_maps = []
    for c in range(NCORES):
        s = slice(c * BC, (c + 1) * BC)
        h0 = hidden[s].astype(f16)                            # [BC, H]
        h0t = h0.T.reshape(JT, 128, BC).transpose(1, 0, 2).reshape(128, JT * BC)
        x0 = np.concatenate(
            [initial_input[s, 0][None, :], np.ones((1, BC))], axis=0
        ).astype(f16)                                         # [2, BC]
        tgt = targets[:, s, 0].reshape(1, T * BC).astype(f16)  # [1, T*BC]
        m = dict(shared)
        m.update(h0t=np.ascontiguousarray(h0t), x0t=x0,
                 tgt16=np.ascontiguousarray(tgt))
        in_maps.append(m)
    return in_maps


def kernel(initial_input, hidden, targets, W_ih, b_ih, W_hh, b_hh,
           W_out, b_out, tf_mask):
    from concourse.bass_utils import run_bass_kernel_spmd

    if "nc" not in _CACHE:
        _CACHE["nc"] = _build_program(T)
    nc = _CACHE["nc"]

    in_maps = _prep_inputs(initial_input, hidden, targets, W_ih, b_ih,
                           W_hh, b_hh, W_out, b_out, tf_mask)
    res = run_bass_kernel_spmd(nc, in_maps, list(range(NCORES)))
    # y_out per core: [1, T*BC] -> full output [T, B, 1]
    ys = [res.results[c]["y_out"].reshape(T, BC) for c in range(NCORES)]
    out = np.concatenate([y[:, :, None] for y in ys], axis=1)
    return np.ascontiguousarray(out.astype(np.float32))

